# revision 27
# baseline (speedup 1.0000x reference)
"""Trainium2 Bass kernel for nn_AttentionBlock (64, 512, 16) / three 8192x8192 Linears.

Strategy (8 NeuronCores, single NEFF, one launch):
  Projections (column-sharded, fp8 DoubleRow): core c owns output cols
    [1024c, 1024(c+1)) of each Linear. Weights pre-transposed, pre-scaled
    by 64, pre-swizzled into contiguous 2MB chunks; q/k/v chunks stream
    round-robin on the two HWDGE rings from t=0 so all three projections
    finish together at the ~358GB/s per-core HBM floor.
  Exchange: ONE AllToAll carries q,k AND v in a single 192KB payload
    (block for peer j = [its 8 batches, (q/k: t,d,w | v: w,d)]). The
    first collective after the runtime's mesh-init barrier pays a large
    one-time warm-up cost; merging everything into that single first
    collective means the whole exchange completes right behind the
    barrier, and the attention tail starts as early as the fabric allows.
  Attention (batch-sharded): alphas built transposed [kw, qw] so softmax
    over the query axis is a free-dim reduction. Wide [128,2,512] exps on
    ACT, denominator row-sums on DVE, exp'd alphas stay bf16 and feed the
    second einsum directly; the softmax reciprocal (x256 for range) folds
    into the tiny v tiles. Post-collective gathers are spread across the
    sync/scalar/gpsimd queues so the first batch's attention starts ~1us
    after the exchange lands.
  Part B: per output group of 4 batches, 16 column-tiled matmuls
    accumulate into one packed [128,512] PSUM bank; sigmoid is computed
    as 1/(1+exp(-x)) on ACT+DVE so the single-slot activation table never
    reloads; residual add on DVE, eight 32KB output DMAs.
  Host: gathers per-core (128, 512) outputs, transposes back.
"""

import math

import numpy as np
import ml_dtypes

import concourse.bass as bass
import concourse.bacc as bacc
import concourse.mybir as mybir
import concourse.tile as tile
import concourse.bass_utils as bass_utils

N_CORES = 8
BS, W_DIM, D = 64, 512, 16
K = W_DIM * D            # 8192 contraction dim
CPC = K // N_CORES       # 1024 output cols per core
WPC = W_DIM // N_CORES   # 64 w positions per core
BPC = BS // N_CORES      # 8 batches per core
NKT = K // 128           # 64 k-tiles
CHUNK = 16               # k-tiles per weight DMA (2MB chunks)
NCH = NKT // CHUNK       # 4 weight chunks per tensor
WSCALE = 64.0            # host-side weight pre-scale (fp8 subnormal dodge)
QKSCALE = 8.0            # q/k payload post-scale divisor (fp8 range fit)
EASCALE = 256.0          # folded into vt (softmax-recip path) for range
EXP_SCALE = QKSCALE * QKSCALE / (math.sqrt(K) * WSCALE * WSCALE)
BLK = 2 * D * WPC + WPC * D   # 3072: per-batch exchange payload bytes

_CACHE: dict = {}


def _build():
    f8 = mybir.dt.float8e4
    bf16 = mybir.dt.bfloat16
    f32 = mybir.dt.float32
    DR = mybir.MatmulPerfMode.DoubleRow
    Exp = mybir.ActivationFunctionType.Exp

    nc = bacc.Bacc("TRN2", target_bir_lowering=False, debug=False,
                   num_devices=N_CORES)

    xt_d = nc.dram_tensor("xt", [128, NKT * BS], f8, kind="ExternalInput")
    w_d = [nc.dram_tensor(n, [NCH * 128, CHUNK * CPC], f8,
                          kind="ExternalInput")
           for n in ("wq", "wk", "wv")]
    b_d = [nc.dram_tensor(n, [1, CPC], bf16, kind="ExternalInput")
           for n in ("bq", "bk", "bv")]
    xtp_d = nc.dram_tensor("xtp", [2 * 128, W_DIM], f32, kind="ExternalInput")
    out_d = nc.dram_tensor("out", [2 * 128, W_DIM], f32,
                           kind="ExternalOutput")

    with tile.TileContext(nc) as tc:
        with (
            tc.tile_pool(name="constp", bufs=1) as constp,
            tc.tile_pool(name="sbp", bufs=1) as sbp,
            tc.tile_pool(name="dramp", bufs=1, space="DRAM") as dramp,
            tc.tile_pool(name="wpa", bufs=3) as wpa,
            tc.tile_pool(name="wpb", bufs=3) as wpb,
        ):
            # ---- mesh warm-up: a throwaway collective triggered first.
            # The first collective after the runtime mesh-init barrier pays
            # a ~30us warm-up that scales with payload; burning it on 8
            # bytes here means the real exchange below runs at warm speed.
            warm = constp.tile([1, N_CORES], f8, name="warm")
            nc.gpsimd.memset(warm[:], 0.0)
            warm_in = dramp.tile([N_CORES, 1], f8, tag="warm_in",
                                 name="warm_in")
            warm_out = dramp.tile([N_CORES, 1], f8, tag="warm_out",
                                  name="warm_out")
            nc.gpsimd.dma_start(warm_in.rearrange("j x -> x j"), warm[:])
            nc.gpsimd.collective_compute(
                "AllToAll", mybir.AluOpType.bypass,
                replica_groups=[list(range(N_CORES))],
                ins=[warm_in.opt()], outs=[warm_out.opt()])

            # ---- constants / inputs (gpsimd queue; weight rings stay free)
            xt_sb = constp.tile([128, NKT, BS], f8)
            nc.gpsimd.dma_start(
                xt_sb[:], xt_d[:, :].rearrange("p (kt b) -> p kt b", kt=NKT))
            xtp_sb = constp.tile([128, 2, W_DIM], f32)
            nc.gpsimd.dma_start(
                xtp_sb[:], xtp_d[:, :].rearrange("(g p) w -> p g w", g=2))
            ones = constp.tile([1, BS], bf16)
            nc.vector.memset(ones[:], 1.0)
            b_sb = []
            for t in range(3):
                bt = constp.tile([1, CPC], bf16, name=f"bias{t}")
                nc.gpsimd.dma_start(bt[:], b_d[t][:, :])
                b_sb.append(bt)
            # warm the single-slot ScalarE activation table with Exp
            tbl = constp.tile([1, 8], f32)
            nc.scalar.activation(tbl[:], ones[:, 0:8], Exp)

            qk_sb = sbp.tile([BS, 2, D, WPC], f8, name="qk_sb")
            v_sb = sbp.tile([BS, CPC], f8, name="v_sb")
            a2a1_in = dramp.tile([N_CORES, BPC, 2, D, WPC], f8,
                                 tag="a2a1_in", name="a2a1_in")
            a2a1_out = dramp.tile([N_CORES, BPC, 2, D, WPC], f8,
                                  tag="a2a1_out", name="a2a1_out")
            a2a2_in = dramp.tile([N_CORES, BPC, CPC], f8,
                                 tag="a2a2_in", name="a2a2_in")
            a2a2_out = dramp.tile([N_CORES, BPC, CPC], f8,
                                  tag="a2a2_out", name="a2a2_out")

            hwdge = [nc.sync, nc.scalar]

            def w_chunk_matmuls(m, psum, wt):
                for jj in range(0, CHUNK, 2):
                    for h in range(2):
                        nc.tensor.matmul(
                            psum[:, h * 512:(h + 1) * 512],
                            xt_sb[:, CHUNK * m + jj:CHUNK * m + jj + 2, :],
                            wt[:, jj:jj + 2, h * 512:(h + 1) * 512],
                            start=(m == 0 and jj == 0), stop=False,
                            perf_mode=DR)

            def bias_matmuls(t, psum):
                for h in range(2):
                    nc.tensor.matmul(
                        psum[:, h * 512:(h + 1) * 512],
                        ones[:],
                        b_sb[t][:, h * 512:(h + 1) * 512],
                        start=False, stop=True)

            # ---- phase QK: q,k chunks on both rings; A2A1 triggers the
            # moment the payload is staged (rides right behind the warm
            # collective's completion) ----
            with tc.tile_pool(name="qkps", bufs=1, space="PSUM") as qkps:
                psA = [qkps.tile([BS, CPC], f32, name=f"ps{t}")
                       for t in range(2)]
                for m in range(NCH):
                    for t in range(2):
                        wt = [wpa, wpb][t].tile([128, CHUNK, CPC], f8,
                                                tag=f"w{t}", name="wt")
                        hwdge[t].dma_start(
                            wt[:],
                            w_d[t][128 * m:128 * (m + 1), :].rearrange(
                                "p (j c) -> p j c", j=CHUNK))
                        w_chunk_matmuls(m, psA[t], wt)
                for t in range(2):
                    bias_matmuls(t, psA[t])
                    nc.vector.tensor_scalar_mul(
                        qk_sb[:, t, :, :],
                        psA[t].rearrange("b (w d) -> b d w", d=D),
                        1.0 / QKSCALE)
            nc.gpsimd.dma_start(
                a2a1_in.rearrange("j b t d w -> (j b) t d w"),
                qk_sb[:, :, :, :])
            nc.gpsimd.collective_compute(
                "AllToAll", mybir.AluOpType.bypass,
                replica_groups=[list(range(N_CORES))],
                ins=[a2a1_in.opt()], outs=[a2a1_out.opt()])

            # ---- phase V ----
            with tc.tile_pool(name="vps", bufs=1, space="PSUM") as vps:
                psV = vps.tile([BS, CPC], f32, name="psv")
                for m in range(NCH):
                    wt = [wpa, wpb][m % 2].tile([128, CHUNK, CPC], f8,
                                                tag=f"w{m % 2}", name="wt2")
                    hwdge[m % 2].dma_start(
                        wt[:],
                        w_d[2][128 * m:128 * (m + 1), :].rearrange(
                            "p (j c) -> p j c", j=CHUNK))
                    w_chunk_matmuls(m, psV, wt)
                bias_matmuls(2, psV)
                nc.vector.tensor_scalar_mul(v_sb[:], psV[:], 1.0 / WSCALE)
            nc.gpsimd.dma_start(
                a2a2_in.rearrange("j b c -> (j b) c"), v_sb[:])
            nc.gpsimd.collective_compute(
                "AllToAll", mybir.AluOpType.bypass,
                replica_groups=[list(range(N_CORES))],
                ins=[a2a2_in.opt()], outs=[a2a2_out.opt()])

            qk_region = a2a1_out
            v_region = a2a2_out.rearrange("i b (w d) -> i b w d", d=D)

            with (
                tc.tile_pool(name="attps", bufs=1, space="PSUM") as attps,
                tc.tile_pool(name="keepp", bufs=1) as keepp,
            ):
                # qkT gathers all on the SYNC ring: the scalar/ACT queue
                # must stay empty so the exp chain starts the moment the
                # first batch's data lands (ACT is in-order with its DMAs)
                qkT = keepp.tile([D, 2, BPC, N_CORES, WPC], f8, name="qkT")
                for b in range(BPC):
                    for t, eng in ((0, nc.sync), (1, nc.gpsimd)):
                        eng.dma_start(
                            qkT[:, t, b, :, :],
                            qk_region[:, b, t, :, :].rearrange(
                                "i d w -> d i w"))
                # vt gathers split sync/gpsimd (sync is free after qkT)
                vt_all = keepp.tile([128, BPC, 4, D], f8, name="vt_all")
                for Kb in range(4):
                    for h2 in range(2):
                        eng = nc.sync if Kb < 2 else nc.gpsimd
                        eng.dma_start(
                            vt_all[64 * h2:64 * h2 + 64, :, Kb, :],
                            v_region[2 * Kb + h2, :, :, :].rearrange(
                                "b w d -> w b d"))

                den_tiles = [keepp.tile([128, 4], f32, tag=f"den{b}",
                                        name=f"den{b}") for b in range(BPC)]
                ea_tiles = {}
                vt2_tiles = []
                # per batch-pair: matmuls -> wide exp (ACT) -> den (DVE)
                # -> per-pair reciprocal (DVE) -> vt2 scale on GPSIMD.
                # DVE carries only den+rec (under the exp rate); the vt2
                # scales ride the otherwise-idle Pool queue, and splitting
                # rec per pair lets the last group's Kb0/1 matmuls run
                # while the final denominator is still reducing.
                for b in range(BPC):
                    rec = keepp.tile([128, 4], f32, tag=f"rec{b}",
                                     name=f"rec{b}")
                    vt2 = keepp.tile([128, 4, D], bf16, tag=f"vt2{b}",
                                     name=f"vt2{b}")
                    for j in range(2):
                        aT2 = attps.tile([128, 2, 512], f32, tag="aT2",
                                         name="aT2", bufs=3)
                        for h in range(2):
                            kw = 2 * j + h
                            nc.tensor.matmul(
                                aT2[:, h, :],
                                qkT[:, 1, b, 2 * kw:2 * kw + 2, :],
                                qkT[:, 0, b, :, :],
                                start=True, stop=True)
                        ea = keepp.tile([128, 2, 512], bf16,
                                        tag=f"ea{b}_{j}", name=f"ea{b}_{j}")
                        nc.scalar.activation(ea[:], aT2[:], Exp,
                                             scale=EXP_SCALE)
                        nc.vector.tensor_reduce(
                            den_tiles[b][:, 2 * j:2 * j + 2], ea[:],
                            axis=mybir.AxisListType.X,
                            op=mybir.AluOpType.add)
                        ea_tiles[(b, j)] = ea
                        nc.vector.reciprocal(
                            rec[:, 2 * j:2 * j + 2],
                            den_tiles[b][:, 2 * j:2 * j + 2])
                        for Kb in (2 * j, 2 * j + 1):
                            nc.gpsimd.tensor_scalar(
                                vt2[:, Kb, :], vt_all[:, b, Kb, :],
                                rec[:, Kb:Kb + 1], EASCALE,
                                op0=mybir.AluOpType.mult,
                                op1=mybir.AluOpType.mult)
                    vt2_tiles.append(vt2)

                # Pre-warm the single-slot ACT table with Sigmoid: queued
                # right after the last exp, the ~1.5us table load runs
                # during the barrier wait instead of on the post-fence tail.
                tbl2 = keepp.tile([1, 8], f32, name="tbl2")
                nc.scalar.activation(
                    tbl2[:], ones[:, 0:8],
                    mybir.ActivationFunctionType.Sigmoid)

                # Fence part A / part B: without it Tile interleaves the
                # attn-B matmuls (which wait on vt2 <- vt gathers <- A2A2)
                # into the attn-A MM stream, head-of-line blocking the
                # remaining alpha matmuls ~10us while the v exchange lands.
                # (A no_sync_barrier variant measured WORSE: exp cadence
                # degraded 996->1196ns from the altered sync-edge layout.)
                tc.strict_bb_all_engine_barrier()

                # ---- part B: column-tiled packed matmuls + sigmoid ----
                for r in range(2):
                    rT = attps.tile([128, W_DIM], f32, tag="rT", name="rT",
                                    bufs=2)
                    for Kb in range(4):
                        for g in range(4):
                            b = 4 * r + g
                            nc.tensor.matmul(
                                rT[32 * g:32 * g + D, :],
                                vt2_tiles[b][:, Kb, :],
                                ea_tiles[(b, Kb // 2)][:, Kb % 2, :],
                                start=(Kb == 0), stop=(Kb == 3),
                                tile_position=(0, 32 * g))
                    # native ACT sigmoid: one table load after the last exp
                    # beats DVE 1/(1+e) -- DVE reciprocal is ~3.3us/tile
                    sp = keepp.tile([128, W_DIM], f32, tag=f"sp{r}",
                                    name=f"sp{r}")
                    nc.scalar.activation(
                        sp[:], rT[:],
                        mybir.ActivationFunctionType.Sigmoid,
                        scale=1.0 / EASCALE)
                    sg = keepp.tile([128, W_DIM], f32, tag=f"sg{r}",
                                    name=f"sg{r}")
                    nc.vector.tensor_tensor(
                        sg[:], sp[:], xtp_sb[:, r, :],
                        op=mybir.AluOpType.add)
                    # one fat DMA per group (padding rows included; host
                    # strips them) instead of four quadrant DMAs
                    hwdge[r].dma_start(
                        out_d[128 * r:128 * (r + 1), :], sg[:])

    nc.compile()
    return nc


def _prep_in_maps(x_in, Wq, bq, Wk, bk, Wv, bv):
    f8 = ml_dtypes.float8_e4m3
    bf16 = ml_dtypes.bfloat16
    x_flat = np.ascontiguousarray(np.asarray(x_in, np.float32).reshape(BS, K))
    xt = np.ascontiguousarray(
        x_flat.T.reshape(NKT, 128, BS).transpose(1, 0, 2)
    ).reshape(128, NKT * BS).astype(f8)
    ws = [np.ascontiguousarray(np.asarray(W, np.float32).T) * WSCALE
          for W in (Wq, Wk, Wv)]
    bs = [(np.asarray(b, np.float32) * WSCALE).reshape(1, K).astype(bf16)
          for b in (bq, bk, bv)]
    xtp = np.ascontiguousarray(
        np.asarray(x_in, np.float32).transpose(0, 2, 1))       # (BS, D, W)

    in_maps = []
    for c in range(N_CORES):
        cs = slice(CPC * c, CPC * (c + 1))
        m = {"xt": xt}
        for nm, w in zip(("wq", "wk", "wv"), ws):
            m[nm] = np.ascontiguousarray(
                w[:, cs].reshape(NCH, CHUNK, 128, CPC).transpose(0, 2, 1, 3)
            ).reshape(NCH * 128, CHUNK * CPC).astype(f8)
        for nm, b in zip(("bq", "bk", "bv"), bs):
            m[nm] = np.ascontiguousarray(b[:, cs])
        xp = np.zeros((2, 4, 32, W_DIM), np.float32)
        xp[:, :, :D, :] = xtp[BPC * c:BPC * (c + 1)].reshape(2, 4, D, W_DIM)
        m["xtp"] = xp.reshape(2 * 128, W_DIM)
        in_maps.append(m)
    return in_maps


def _assemble(results):
    out = np.empty((BS, W_DIM, D), np.float32)
    for c in range(N_CORES):
        # [2 groups, 4 quadrants, 32 rows (16 d + 16 pad), W] -> batches
        o = results[c]["out"].reshape(2, 4, 32, W_DIM)[:, :, :D, :]
        out[BPC * c:BPC * (c + 1)] = o.reshape(BPC, D, W_DIM).transpose(
            0, 2, 1)
    return out


def get_nc():
    if "nc" not in _CACHE:
        _CACHE["nc"] = _build()
    return _CACHE["nc"]


def kernel(x_in, Wq, bq, Wk, bk, Wv, bv):
    nc = get_nc()
    in_maps = _prep_in_maps(x_in, Wq, bq, Wk, bk, Wv, bv)
    res = bass_utils.run_bass_kernel_spmd(
        nc, in_maps, core_ids=list(range(N_CORES)))
    return _assemble(res.results)
